# revision 7
# baseline (speedup 1.0000x reference)
"""CFR network (moe_routing) Trainium2 Bass kernel.

Strategy:
  - Pure data parallel over 8 NeuronCores; MoE routing (treat/control expert
    selection) is done host-side by stable-sorting rows on t so each core
    processes a contiguous treat block then a contiguous control block and
    only ever runs ONE expert MLP per row (halves expert compute vs the
    dense reference).
  - Feature-major activations on chip: weights are the stationary matmul
    operand, activations stream as rhs [features(K) x batch(N)].  Everything
    is fp16 (matmuls accumulate fp32 into PSUM).
  - Layers carry h' = elu(z)+1 (the -1 is folded into the next layer's bias,
    b_eff = b - W @ 1), computed exactly as
        e  = Exp(z + b)            ScalarE, PSUM read, fused bias
        u  = min(e, 1)             DVE 4x / GpSimd, SBUF fp16
        h' = (z + (b+1)) max u     DVE scalar_tensor_tensor, PSUM read
  - Emission is software-pipelined: layers of a wave of supertiles are
    interleaved so each in-order engine always has independent work.
  - The r output is stored as h' (= r+1, fp16); the host subtracts 1.
"""

import math
from contextlib import ExitStack

import numpy as np

B = 262144
FEAT = 128
REP = 200
HYP = 200
NCORES = 8
QUANT = 1024  # batch columns per supertile
NH = 512      # matmul free-dim per instruction (one PSUM bank of fp32)

_F16 = np.float16

_LAYERS = ["r0", "r1", "r2", "t0", "t1", "t2", "c0", "c1", "c2"]


def _pack_host(inputs):
    """Transpose/cast weights, fold the +1 carry into biases, pack into two
    flat arrays (one fp16 weight pack, one fp32 bias pack)."""
    ws = {}
    bs = {}
    for name in _LAYERS:
        w = np.asarray(inputs[f"w_{name[0]}{name[1]}"], np.float32)
        b = np.asarray(inputs[f"b_{name[0]}{name[1]}"], np.float32)
        if name != "r0":
            b = b - w.sum(axis=1)
        ws[name] = w.T.astype(_F16)  # [d_in, d_out] = lhsT
        bs[name] = b

    w_o = np.asarray(inputs["w_o"], np.float32)
    b_o = np.asarray(inputs["b_o"], np.float32)
    b_o_eff = float(b_o[0] - w_o.sum())
    wsT_o = w_o.T.astype(_F16)  # [200, 1]

    cols = []
    wcols = {}
    off = 0

    def add(name, arr):
        nonlocal off
        k, m = arr.shape
        pad = np.zeros((128, m), _F16)
        pad[:k] = arr
        cols.append(pad)
        wcols[name] = (off, k, m)
        off += m

    add("r0", ws["r0"])
    for name in _LAYERS[1:]:
        add(name + "_hi", ws[name][:128])
        add(name + "_lo", ws[name][128:])
    add("o_hi", wsT_o[:128])
    add("o_lo", wsT_o[128:])
    wpack = np.concatenate(cols, axis=1)

    bcols = []
    bcol_idx = {}
    for name in _LAYERS:
        b = bs[name]
        for chunk, sl in (("hi", slice(0, 128)), ("lo", slice(128, 200))):
            for var, delta in (("e", 0.0), ("c", 1.0)):
                v = np.zeros((128,), np.float32)
                seg = b[sl] + delta
                v[: len(seg)] = seg
                bcol_idx[(name, chunk, var)] = len(bcols)
                bcols.append(v)
    v = np.zeros((128,), np.float32)
    v[0] = b_o_eff
    bcol_idx[("o", "hi", "e")] = len(bcols)
    bcols.append(v)
    bpack = np.stack(bcols, axis=1).astype(np.float32)

    return wpack, wcols, bpack, bcol_idx, b_o_eff


def _build_program(R, n_treat_tiles, wcols, bcol_idx, b_o_eff, wcols_total, nb,
                   reps=1, wave=3,
                   umin_pool=lambda li, mi: li % 3 != 0):
    """Single-core Bass/Tile program (shared SPMD across the 8 cores).

    reps>1 repeats the whole computation inside the NEFF (timing only)."""
    import concourse.bass as bass
    import concourse.tile as tile
    from concourse import bacc, mybir

    fp16 = mybir.dt.float16
    f32 = mybir.dt.float32
    Alu = mybir.AluOpType
    Act = mybir.ActivationFunctionType

    nc = bacc.Bacc("TRN2", target_bir_lowering=False, debug=False,
                   enable_asserts=False)

    x_d = nc.dram_tensor("xT", [FEAT, R], fp16, kind="ExternalInput")
    w_d = nc.dram_tensor("wpack", [128, wcols_total], fp16, kind="ExternalInput")
    b_d = nc.dram_tensor("bpack", [128, nb], f32, kind="ExternalInput")
    r_d = nc.dram_tensor("rT", [REP, R], fp16, kind="ExternalOutput")
    y_d = nc.dram_tensor("yf", [1, R], f32, kind="ExternalOutput")

    n_tiles = R // QUANT
    NHALVES = QUANT // NH

    with tile.TileContext(nc) as tc, ExitStack() as ctx:
        wpool = ctx.enter_context(tc.tile_pool(name="w", bufs=1))
        xpool = ctx.enter_context(tc.tile_pool(name="x", bufs=6))
        hpool = ctx.enter_context(tc.tile_pool(name="h", bufs=4))
        epool = ctx.enter_context(tc.tile_pool(name="e", bufs=4))
        opool = ctx.enter_context(tc.tile_pool(name="o", bufs=4))
        pshi = ctx.enter_context(
            tc.tile_pool(name="pshi", bufs=2, space=bass.MemorySpace.PSUM))
        pslo = ctx.enter_context(
            tc.tile_pool(name="pslo", bufs=2, space=bass.MemorySpace.PSUM))

        wt = wpool.tile([128, wcols_total], fp16, tag="wt")
        nc.sync.dma_start(wt[:], w_d[:])
        bt = wpool.tile([128, nb], f32, tag="bt")
        nc.sync.dma_start(bt[:], b_d[:])

        def bias_ap(name, chunk, var, parts):
            c = bcol_idx[(name, chunk, var)]
            return bt[0:parts, c:c + 1]

        def load_x(s):
            col0 = s * QUANT
            xt = xpool.tile([FEAT, QUANT], fp16, tag="x")
            nc.sync.dma_start(xt[:], x_d[:, col0:col0 + QUANT])
            return [(xt, FEAT)]

        def emit_layer(s, li, cur):
            expert = "t" if s < n_treat_tiles else "c"
            lname = ["r0", "r1", "r2", expert + "0", expert + "1",
                     expert + "2"][li]
            col0 = s * QUANT
            wchunks = ["r0"] if lname == "r0" else [lname + "_hi", lname + "_lo"]
            new = []
            for mi, (mc, m0, chunk, pool) in enumerate(
                    ((128, 0, "hi", pshi), (72, 128, "lo", pslo))):
                z = pool.tile([mc, QUANT], f32, tag=f"z{mi}")
                for hh in range(NHALVES):
                    nsl = slice(hh * NH, (hh + 1) * NH)
                    for ki, (rt, kc) in enumerate(cur):
                        off, k, m = wcols[wchunks[ki]]
                        lhsT = wt[0:k, off + m0: off + m0 + mc]
                        nc.tensor.matmul(z[:, nsl], lhsT, rt[:, nsl],
                                         start=(ki == 0),
                                         stop=(ki == len(cur) - 1))
                et = epool.tile([mc, QUANT], fp16, tag=f"e{mi}")
                nc.scalar.activation(et[:], z[:], Act.Exp,
                                     bias=bias_ap(lname, chunk, "e", mc))
                ut = epool.tile([mc, QUANT], fp16, tag=f"u{mi}")
                if umin_pool(li, mi):
                    nc.gpsimd.tensor_scalar(ut[:], et[:], 1.0, None, Alu.min)
                else:
                    nc.vector.tensor_scalar(ut[:], et[:], 1.0, None, Alu.min)
                ht = hpool.tile([mc, QUANT], fp16, tag=f"h{mi}")
                nc.vector.scalar_tensor_tensor(
                    ht[:], z[:], bias_ap(lname, chunk, "c", mc), ut[:],
                    Alu.add, Alu.max)
                new.append((ht, mc))
            if lname == "r2":
                # store r+1 (fp16); host subtracts the 1
                for mi, (ht, mc) in enumerate(new):
                    nc.sync.dma_start(
                        r_d[mi * 128:mi * 128 + mc, col0:col0 + QUANT], ht[:])
            return new

        def emit_yf(s, cur):
            col0 = s * QUANT
            yo = opool.tile([1, QUANT], f32, tag="y")
            for hh in range(NHALVES):
                nsl = slice(hh * NH, (hh + 1) * NH)
                zy = pshi.tile([1, NH], f32, tag="z0")
                for ki, (rt, kc) in enumerate(cur):
                    off, k, m = wcols["o_hi" if ki == 0 else "o_lo"]
                    nc.tensor.matmul(zy[:], wt[0:k, off:off + 1], rt[:, nsl],
                                     start=(ki == 0), stop=(ki == len(cur) - 1))
                nc.scalar.activation(yo[:, nsl], zy[:], Act.Identity,
                                     bias=bias_ap("o", "hi", "e", 1))
            nc.sync.dma_start(y_d[:, col0:col0 + QUANT], yo[:])

        def body():
            for w0 in range(0, n_tiles, wave):
                ss = list(range(w0, min(w0 + wave, n_tiles)))
                curs = {s: load_x(s) for s in ss}
                for li in range(6):
                    for s in ss:
                        curs[s] = emit_layer(s, li, curs[s])
                for s in ss:
                    emit_yf(s, curs[s])

        if reps == 1:
            body()
        else:
            with tc.For_i(0, reps, 1):
                body()

    nc.compile()
    return nc


def _route(t):
    """Host-side MoE routing: stable sort by t, even split over cores,
    pad each core's treat/control block up to a QUANT multiple."""
    order = np.argsort(t, kind="stable")
    n_t = int((t == 0).sum())
    n_c = t.shape[0] - n_t

    def cap(n):
        if n == 0:
            return 0
        return int(math.ceil(n / (NCORES * QUANT))) * QUANT

    NT, NC = cap(n_t), cap(n_c)
    tchunks, cchunks = [], []
    for c in range(NCORES):
        lo = min(c * NT, n_t)
        hi = min((c + 1) * NT, n_t)
        tchunks.append(order[lo:hi])
        lo = min(c * NC, n_c)
        hi = min((c + 1) * NC, n_c)
        cchunks.append(order[n_t + lo:n_t + hi])
    return NT, NC, tchunks, cchunks


def kernel(**inputs):
    x = np.asarray(inputs["x"], np.float32)
    t = np.asarray(inputs["t"]).astype(np.int64)
    assert x.shape == (B, FEAT)

    wpack, wcols, bpack, bcol_idx, b_o_eff = _pack_host(inputs)
    NT, NC, tchunks, cchunks = _route(t)
    R = NT + NC

    x16 = x.astype(_F16)
    in_maps = []
    for c in range(NCORES):
        xc = np.zeros((FEAT, R), _F16)
        tc_, cc_ = tchunks[c], cchunks[c]
        if len(tc_):
            xc[:, :len(tc_)] = x16[tc_].T
        if len(cc_):
            xc[:, NT:NT + len(cc_)] = x16[cc_].T
        in_maps.append({"xT": np.ascontiguousarray(xc),
                        "wpack": wpack, "bpack": bpack})

    nc = _build_program(R, NT // QUANT, wcols, bcol_idx, b_o_eff,
                        wpack.shape[1], bpack.shape[1])

    from concourse import bass_utils
    res = bass_utils.run_bass_kernel_spmd(
        nc, in_maps, list(range(NCORES))).results

    r_full = np.empty((B, REP), np.float32)
    yf_full = np.empty((B, 1), np.float32)
    for c in range(NCORES):
        rT = res[c]["rT"]
        yf = res[c]["yf"]
        tc_, cc_ = tchunks[c], cchunks[c]
        if len(tc_):
            r_full[tc_] = rT[:, :len(tc_)].T.astype(np.float32) - 1.0
            yf_full[tc_, 0] = yf[0, :len(tc_)]
        if len(cc_):
            r_full[cc_] = rT[:, NT:NT + len(cc_)].T.astype(np.float32) - 1.0
            yf_full[cc_, 0] = yf[0, NT:NT + len(cc_)]
    return (r_full, yf_full)


if __name__ == "__main__":
    import reference

    inputs = {k: np.asarray(v) for k, v in reference.setup_inputs().items()}
    out = kernel(**inputs)
    print("r", out[0].shape, out[0].dtype, "yf", out[1].shape, out[1].dtype)


# revision 19
# speedup vs baseline: 4.4507x; 4.4507x over previous
"""CFR network (moe_routing) Trainium2 Bass kernel.

Strategy:
  - Pure data parallel over 8 NeuronCores; MoE routing (treat/control expert
    selection) is done host-side by stable-sorting rows on t so each core
    processes a contiguous treat block then a contiguous control block and
    only ever runs ONE expert MLP per row (halves expert compute vs the
    dense reference).
  - Feature-major activations on chip: weights are the stationary matmul
    operand, activations stream as rhs [features(K) x batch(N)].  Everything
    is fp16 (matmuls accumulate fp32 into PSUM).
  - Layers carry h' = elu(z)+1 (the -1 is folded into the next layer's bias,
    b_eff = b - W @ 1), computed exactly as
        e  = Exp(z + b)            ScalarE, PSUM read, fused bias
        u  = min(e, 1)             DVE 4x / GpSimd, SBUF fp16
        h' = (z + (b+1)) max u     DVE scalar_tensor_tensor, PSUM read
  - Emission is software-pipelined: layers of a wave of supertiles are
    interleaved so each in-order engine always has independent work.
  - The r output is stored as h' (= r+1, fp16); the host subtracts 1.
"""

import math
from contextlib import ExitStack

import numpy as np

B = 262144
FEAT = 128
REP = 200
HYP = 200
NCORES = 8
QUANT = 1024  # batch columns per supertile
NH = 512      # matmul free-dim per instruction (one PSUM bank of fp32)

_F16 = np.float16

_LAYERS = ["r0", "r1", "r2", "t0", "t1", "t2", "c0", "c1", "c2"]


def _pack_host(inputs):
    """Transpose/cast weights, fold the +1 carry into biases, pack into two
    flat arrays (one fp16 weight pack, one fp32 bias pack)."""
    ws = {}
    bs = {}
    for name in _LAYERS:
        w = np.asarray(inputs[f"w_{name[0]}{name[1]}"], np.float32)
        b = np.asarray(inputs[f"b_{name[0]}{name[1]}"], np.float32)
        if name != "r0":
            b = b - w.sum(axis=1)
        ws[name] = w.T.astype(_F16)  # [d_in, d_out] = lhsT
        bs[name] = b

    w_o = np.asarray(inputs["w_o"], np.float32)
    b_o = np.asarray(inputs["b_o"], np.float32)
    b_o_eff = float(b_o[0] - w_o.sum())
    wsT_o = w_o.T.astype(_F16)  # [200, 1]

    cols = []
    wcols = {}
    off = 0

    def add(name, arr):
        nonlocal off
        k, m = arr.shape
        pad = np.zeros((128, m), _F16)
        pad[:k] = arr
        cols.append(pad)
        wcols[name] = (off, k, m)
        off += m

    add("r0", ws["r0"])
    for name in _LAYERS[1:]:
        add(name + "_hi", ws[name][:128])
        add(name + "_lo", ws[name][128:])
    add("o_hi", wsT_o[:128])
    add("o_lo", wsT_o[128:])
    wpack = np.concatenate(cols, axis=1)

    bcols = []
    bcol_idx = {}
    for name in _LAYERS:
        b = bs[name]
        for chunk, sl in (("hi", slice(0, 128)), ("lo", slice(128, 200))):
            for var, delta in (("e", 0.0), ("c", 1.0)):
                v = np.zeros((128,), np.float32)
                seg = b[sl] + delta
                v[: len(seg)] = seg
                bcol_idx[(name, chunk, var)] = len(bcols)
                bcols.append(v)
    v = np.zeros((128,), np.float32)
    v[0] = b_o_eff
    bcol_idx[("o", "hi", "e")] = len(bcols)
    bcols.append(v)
    bpack = np.stack(bcols, axis=1).astype(np.float32)

    return wpack, wcols, bpack, bcol_idx, b_o_eff


def _build_program(R, n_treat_tiles, wcols, bcol_idx, b_o_eff, wcols_total, nb,
                   reps=1, wave=3,
                   umin_pool=lambda li, mi: False,
                   path_c=lambda li, mi: mi == 1 and li in (1, 3, 5),
                   probe=None):
    """Single-core Bass/Tile program (shared SPMD across the 8 cores).

    reps>1 repeats the whole computation inside the NEFF (timing only)."""
    import concourse.bass as bass
    import concourse.tile as tile
    from concourse import bacc, mybir

    fp16 = mybir.dt.float16
    f32 = mybir.dt.float32
    Alu = mybir.AluOpType
    Act = mybir.ActivationFunctionType

    nc = bacc.Bacc("TRN2", target_bir_lowering=False, debug=False,
                   enable_asserts=False)

    x_d = nc.dram_tensor("xT", [FEAT, R], fp16, kind="ExternalInput")
    w_d = nc.dram_tensor("wpack", [128, wcols_total], fp16, kind="ExternalInput")
    b_d = nc.dram_tensor("bpack", [128, nb], f32, kind="ExternalInput")
    r_d = nc.dram_tensor("rT", [REP, R], fp16, kind="ExternalOutput")
    y_d = nc.dram_tensor("yf", [1, R], f32, kind="ExternalOutput")

    n_tiles = R // QUANT
    NHALVES = QUANT // NH

    with tile.TileContext(nc) as tc, ExitStack() as ctx:
        wpool = ctx.enter_context(tc.tile_pool(name="w", bufs=1))
        xpool = ctx.enter_context(tc.tile_pool(name="x", bufs=6))
        hpool = ctx.enter_context(tc.tile_pool(name="h", bufs=4))
        epool = ctx.enter_context(tc.tile_pool(name="e", bufs=4))
        opool = ctx.enter_context(tc.tile_pool(name="o", bufs=4))
        pshi = ctx.enter_context(
            tc.tile_pool(name="pshi", bufs=2, space=bass.MemorySpace.PSUM))
        pslo = ctx.enter_context(
            tc.tile_pool(name="pslo", bufs=2, space=bass.MemorySpace.PSUM))

        wt = wpool.tile([128, wcols_total], fp16, tag="wt")
        nc.sync.dma_start(wt[:], w_d[:])
        bt = wpool.tile([128, nb], f32, tag="bt")
        nc.sync.dma_start(bt[:], b_d[:])
        dummy = None
        if probe == "noact":
            dummy = []
            for mi, mc in ((0, 128), (1, 72)):
                dtile = wpool.tile([mc, QUANT], fp16, tag=f"dum{mi}")
                nc.vector.memset(dtile[:], 1.0)
                dummy.append(dtile)

        def bias_ap(name, chunk, var, parts):
            c = bcol_idx[(name, chunk, var)]
            return bt[0:parts, c:c + 1]

        def load_x(s):
            col0 = s * QUANT
            xt = xpool.tile([FEAT, QUANT], fp16, tag="x")
            nc.sync.dma_start(xt[:], x_d[:, col0:col0 + QUANT])
            return [(xt, FEAT)]

        def emit_layer(s, li, cur):
            expert = "t" if s < n_treat_tiles else "c"
            lname = ["r0", "r1", "r2", expert + "0", expert + "1",
                     expert + "2"][li]
            col0 = s * QUANT
            wchunks = ["r0"] if lname == "r0" else [lname + "_hi", lname + "_lo"]
            new = []
            for mi, (mc, m0, chunk, pool) in enumerate(
                    ((128, 0, "hi", pshi), (72, 128, "lo", pslo))):
                z = pool.tile([mc, QUANT], f32, tag=f"z{mi}")
                # K-chunk outer so each stationary weight load serves all
                # N-halves (halves the LDWEIGHTS count)
                for ki, (rt, kc) in enumerate(cur):
                    off, k, m = wcols[wchunks[ki]]
                    lhsT = wt[0:k, off + m0: off + m0 + mc]
                    for hh in range(NHALVES):
                        nsl = slice(hh * NH, (hh + 1) * NH)
                        nc.tensor.matmul(z[:, nsl], lhsT, rt[:, nsl],
                                         start=(ki == 0),
                                         stop=(ki == len(cur) - 1))
                if probe == "mmonly":
                    base = cur[0][0]
                    new.append((base[0:mc], mc))
                    continue
                ht = hpool.tile([mc, QUANT], fp16, tag=f"h{mi}")
                if probe == "noact":
                    nc.vector.scalar_tensor_tensor(
                        ht[:], z[:], bias_ap(lname, chunk, "c", mc), dummy[mi][:],
                        Alu.add, Alu.max)
                    new.append((ht, mc))
                    continue
                et = epool.tile([mc, QUANT], fp16, tag=f"e{mi}")
                nc.scalar.activation(et[:], z[:], Act.Exp,
                                     bias=bias_ap(lname, chunk, "e", mc))
                if probe == "nodve":
                    new.append((et, mc))
                    continue
                if probe == "noumin":
                    nc.vector.scalar_tensor_tensor(
                        ht[:], z[:], bias_ap(lname, chunk, "c", mc), et[:],
                        Alu.add, Alu.max)
                    new.append((ht, mc))
                    continue
                ut = epool.tile([mc, QUANT], fp16, tag=f"u{mi}")
                if umin_pool(li, mi):
                    nc.gpsimd.tensor_scalar(ut[:], et[:], 1.0, None, Alu.min)
                else:
                    nc.vector.tensor_scalar(ut[:], et[:], 1.0, None, Alu.min)
                if path_c(li, mi):
                    # ACT extracts the linear branch; DVE combines in fp16 SBUF
                    ct = epool.tile([mc, QUANT], fp16, tag=f"c{mi}")
                    nc.scalar.activation(ct[:], z[:], Act.Identity,
                                         bias=bias_ap(lname, chunk, "c", mc))
                    nc.vector.tensor_tensor(ht[:], ut[:], ct[:], Alu.max)
                else:
                    nc.vector.scalar_tensor_tensor(
                        ht[:], z[:], bias_ap(lname, chunk, "c", mc), ut[:],
                        Alu.add, Alu.max)
                new.append((ht, mc))
            if lname == "r2" and probe != "nostores":
                # store r+1 (fp16); host subtracts the 1
                for mi, (ht, mc) in enumerate(new):
                    nc.sync.dma_start(
                        r_d[mi * 128:mi * 128 + mc, col0:col0 + QUANT], ht[:])
            return new

        def emit_yf(s, cur):
            col0 = s * QUANT
            yo = opool.tile([1, QUANT], f32, tag="y")
            for hh in range(NHALVES):
                nsl = slice(hh * NH, (hh + 1) * NH)
                zy = pshi.tile([1, NH], f32, tag="z0")
                for ki, (rt, kc) in enumerate(cur):
                    off, k, m = wcols["o_hi" if ki == 0 else "o_lo"]
                    nc.tensor.matmul(zy[:], wt[0:k, off:off + 1], rt[:, nsl],
                                     start=(ki == 0), stop=(ki == len(cur) - 1))
                nc.scalar.activation(yo[:, nsl], zy[:], Act.Identity,
                                     bias=bias_ap("o", "hi", "e", 1))
            if probe != "nostores":
                nc.sync.dma_start(y_d[:, col0:col0 + QUANT], yo[:])

        def body():
            for w0 in range(0, n_tiles, wave):
                ss = list(range(w0, min(w0 + wave, n_tiles)))
                curs = {s: load_x(s) for s in ss}
                for li in range(6):
                    for s in ss:
                        curs[s] = emit_layer(s, li, curs[s])
                for s in ss:
                    emit_yf(s, curs[s])

        if reps == 1:
            body()
        else:
            with tc.For_i(0, reps, 1):
                body()

    nc.compile()
    return nc


def _route(t):
    """Host-side MoE routing: stable sort by t, even split over cores,
    pad each core's treat/control block up to a QUANT multiple."""
    order = np.argsort(t, kind="stable")
    n_t = int((t == 0).sum())
    n_c = t.shape[0] - n_t

    def cap(n):
        if n == 0:
            return 0
        return int(math.ceil(n / (NCORES * QUANT))) * QUANT

    NT, NC = cap(n_t), cap(n_c)
    tchunks, cchunks = [], []
    for c in range(NCORES):
        lo = min(c * NT, n_t)
        hi = min((c + 1) * NT, n_t)
        tchunks.append(order[lo:hi])
        lo = min(c * NC, n_c)
        hi = min((c + 1) * NC, n_c)
        cchunks.append(order[n_t + lo:n_t + hi])
    return NT, NC, tchunks, cchunks


def kernel(**inputs):
    x = np.asarray(inputs["x"], np.float32)
    t = np.asarray(inputs["t"]).astype(np.int64)
    assert x.shape == (B, FEAT)

    wpack, wcols, bpack, bcol_idx, b_o_eff = _pack_host(inputs)
    NT, NC, tchunks, cchunks = _route(t)
    R = NT + NC

    x16 = x.astype(_F16)
    in_maps = []
    for c in range(NCORES):
        xc = np.zeros((FEAT, R), _F16)
        tc_, cc_ = tchunks[c], cchunks[c]
        if len(tc_):
            xc[:, :len(tc_)] = x16[tc_].T
        if len(cc_):
            xc[:, NT:NT + len(cc_)] = x16[cc_].T
        in_maps.append({"xT": np.ascontiguousarray(xc),
                        "wpack": wpack, "bpack": bpack})

    nc = _build_program(R, NT // QUANT, wcols, bcol_idx, b_o_eff,
                        wpack.shape[1], bpack.shape[1])

    from concourse import bass_utils
    res = bass_utils.run_bass_kernel_spmd(
        nc, in_maps, list(range(NCORES))).results

    r_full = np.empty((B, REP), np.float32)
    yf_full = np.empty((B, 1), np.float32)
    for c in range(NCORES):
        rT = res[c]["rT"]
        yf = res[c]["yf"]
        tc_, cc_ = tchunks[c], cchunks[c]
        if len(tc_):
            r_full[tc_] = rT[:, :len(tc_)].T.astype(np.float32) - 1.0
            yf_full[tc_, 0] = yf[0, :len(tc_)]
        if len(cc_):
            r_full[cc_] = rT[:, NT:NT + len(cc_)].T.astype(np.float32) - 1.0
            yf_full[cc_, 0] = yf[0, NT:NT + len(cc_)]
    return (r_full, yf_full)


if __name__ == "__main__":
    import reference

    inputs = {k: np.asarray(v) for k, v in reference.setup_inputs().items()}
    out = kernel(**inputs)
    print("r", out[0].shape, out[0].dtype, "yf", out[1].shape, out[1].dtype)


# revision 27
# speedup vs baseline: 6.7067x; 1.5069x over previous
"""CFR network (moe_routing) Trainium2 Bass kernel.

Strategy:
  - Pure data parallel over 8 NeuronCores; MoE routing (treat/control expert
    selection) is done host-side by stable-sorting rows on t so each core
    processes a contiguous treat block then a contiguous control block and
    only ever runs ONE expert MLP per row (halves expert compute vs the
    dense reference).
  - Feature-major activations on chip: weights are the stationary matmul
    operand, activations stream as rhs [features(K) x batch(N)].  Everything
    is fp16 (matmuls accumulate fp32 into PSUM).
  - Layers carry h' = elu(z)+1 (the -1 is folded into the next layer's bias,
    b_eff = b - W @ 1), computed exactly as
        e  = Exp(z + b)            ScalarE, PSUM read, fused bias
        u  = min(e, 1)             DVE 4x / GpSimd, SBUF fp16
        h' = (z + (b+1)) max u     DVE scalar_tensor_tensor, PSUM read
  - Emission is software-pipelined: layers of a wave of supertiles are
    interleaved so each in-order engine always has independent work.
  - The r output is stored as h' (= r+1, fp16); the host subtracts 1.
"""

import math
from contextlib import ExitStack

import numpy as np

B = 262144
FEAT = 128
REP = 200
HYP = 200
NCORES = 8
QUANT = 1024  # batch columns per supertile
NH = 512      # matmul free-dim per instruction (one PSUM bank of fp32)

_F16 = np.float16

_LAYERS = ["r0", "r1", "r2", "t0", "t1", "t2", "c0", "c1", "c2"]


def _pack_host(inputs):
    """Transpose/cast weights, fold the +1 carry into biases, pack into two
    flat arrays (one fp16 weight pack, one fp32 bias pack)."""
    ws = {}
    bs = {}
    for name in _LAYERS:
        w = np.asarray(inputs[f"w_{name[0]}{name[1]}"], np.float32)
        b = np.asarray(inputs[f"b_{name[0]}{name[1]}"], np.float32)
        if name != "r0":
            b = b - w.sum(axis=1)
        ws[name] = w.T.astype(_F16)  # [d_in, d_out] = lhsT
        bs[name] = b

    w_o = np.asarray(inputs["w_o"], np.float32)
    b_o = np.asarray(inputs["b_o"], np.float32)
    b_o_eff = float(b_o[0] - w_o.sum())
    wsT_o = w_o.T.astype(_F16)  # [200, 1]

    cols = []
    wcols = {}
    off = 0

    def add(name, arr):
        nonlocal off
        k, m = arr.shape
        pad = np.zeros((128, m), _F16)
        pad[:k] = arr
        cols.append(pad)
        wcols[name] = (off, k, m)
        off += m

    add("r0", ws["r0"])
    for name in _LAYERS[1:]:
        add(name + "_hi", ws[name][:128])
        # lo K-chunk carries an extra ones-row weight = b_eff + 1, so the
        # matmul output lands in PSUM already biased (z = zb + 1)
        lo = np.concatenate(
            [ws[name][128:], (bs[name] + 1.0)[None, :].astype(_F16)], axis=0)
        add(name + "_lo", lo)
    add("o_hi", wsT_o[:128])
    o_lo = np.concatenate(
        [wsT_o[128:], np.array([[b_o_eff]], _F16)], axis=0)
    add("o_lo", o_lo)
    wpack = np.concatenate(cols, axis=1)

    bcols = []
    bcol_idx = {}
    for name in _LAYERS:
        b = bs[name]
        for chunk, sl in (("hi", slice(0, 128)), ("lo", slice(128, 200))):
            for var, delta in (("e", 0.0), ("c", 1.0)):
                v = np.zeros((128,), np.float32)
                seg = b[sl] + delta
                v[: len(seg)] = seg
                bcol_idx[(name, chunk, var)] = len(bcols)
                bcols.append(v)
    v = np.zeros((128,), np.float32)
    v[0] = b_o_eff
    bcol_idx[("o", "hi", "e")] = len(bcols)
    bcols.append(v)
    bcol_idx[("m1", "hi", "e")] = len(bcols)
    bcols.append(np.full((128,), -1.0, np.float32))
    bpack = np.stack(bcols, axis=1).astype(np.float32)

    return wpack, wcols, bpack, bcol_idx, b_o_eff


def _build_program(R, n_treat_tiles, wcols, bcol_idx, b_o_eff, wcols_total, nb,
                   reps=1, wave=3,
                   umin_pool=lambda li, mi: False,
                   path_c=lambda li, mi: mi == 1 and li in (1, 3, 5),
                   probe=None):
    """Single-core Bass/Tile program (shared SPMD across the 8 cores).

    reps>1 repeats the whole computation inside the NEFF (timing only)."""
    import concourse.bass as bass
    import concourse.tile as tile
    from concourse import bacc, mybir

    fp16 = mybir.dt.float16
    f32 = mybir.dt.float32
    Alu = mybir.AluOpType
    Act = mybir.ActivationFunctionType

    nc = bacc.Bacc("TRN2", target_bir_lowering=False, debug=False,
                   enable_asserts=False)

    x_d = nc.dram_tensor("xT", [FEAT, R], fp16, kind="ExternalInput")
    w_d = nc.dram_tensor("wpack", [128, wcols_total], fp16, kind="ExternalInput")
    b_d = nc.dram_tensor("bpack", [128, nb], f32, kind="ExternalInput")
    r_d = nc.dram_tensor("rT", [REP, R], fp16, kind="ExternalOutput")
    y_d = nc.dram_tensor("yf", [1, R], f32, kind="ExternalOutput")

    n_tiles = R // QUANT
    NHALVES = QUANT // NH

    with tile.TileContext(nc) as tc, ExitStack() as ctx:
        wpool = ctx.enter_context(tc.tile_pool(name="w", bufs=1))
        xpool = ctx.enter_context(tc.tile_pool(name="x", bufs=6))
        hpool = ctx.enter_context(tc.tile_pool(name="h", bufs=4))
        epool = ctx.enter_context(tc.tile_pool(name="e", bufs=4))
        opool = ctx.enter_context(tc.tile_pool(name="o", bufs=4))
        pshi = ctx.enter_context(
            tc.tile_pool(name="pshi", bufs=2, space=bass.MemorySpace.PSUM))
        pslo = ctx.enter_context(
            tc.tile_pool(name="pslo", bufs=2, space=bass.MemorySpace.PSUM))

        wt = wpool.tile([128, wcols_total], fp16, tag="wt")
        nc.sync.dma_start(wt[:], w_d[:])
        bt = wpool.tile([128, nb], f32, tag="bt")
        nc.sync.dma_start(bt[:], b_d[:])
        dummy = None
        if probe == "noact":
            dummy = []
            for mi, mc in ((0, 128), (1, 72)):
                dtile = wpool.tile([mc, QUANT], fp16, tag=f"dum{mi}")
                nc.vector.memset(dtile[:], 1.0)
                dummy.append(dtile)

        # manually rotated lo-activation tiles [73, QUANT]: row 72 is a
        # persistent 1.0 (feeds the bias ones-row of the next matmul)
        NLO = 6
        hlo_tiles = []
        for i in range(NLO):
            hlo = wpool.tile([73, QUANT], fp16, tag=f"hlo{i}")
            # engine APs need a 32-aligned partition base: set rows 64-72 to
            # 1.0; rows 64-71 are rewritten by every stt, row 72 persists
            nc.vector.memset(hlo[64:73, :], 1.0)
            hlo_tiles.append(hlo)
        hlo_ctr = [0]

        def bias_ap(name, chunk, var, parts):
            c = bcol_idx[(name, chunk, var)]
            return bt[0:parts, c:c + 1]

        def load_x(s):
            col0 = s * QUANT
            xt = xpool.tile([FEAT, QUANT], fp16, tag="x")
            nc.sync.dma_start(xt[:], x_d[:, col0:col0 + QUANT])
            return [(xt, FEAT)]

        def emit_layer(s, li, cur):
            expert = "t" if s < n_treat_tiles else "c"
            lname = ["r0", "r1", "r2", expert + "0", expert + "1",
                     expert + "2"][li]
            col0 = s * QUANT
            wchunks = ["r0"] if lname == "r0" else [lname + "_hi", lname + "_lo"]
            new = []
            for mi, (mc, m0, chunk, pool) in enumerate(
                    ((128, 0, "hi", pshi), (72, 128, "lo", pslo))):
                z = pool.tile([mc, QUANT], f32, tag=f"z{mi}")
                # K-chunk outer so each stationary weight load serves all
                # N-halves (halves the LDWEIGHTS count)
                for ki, (rt, kc) in enumerate(cur):
                    off, k, m = wcols[wchunks[ki]]
                    lhsT = wt[0:k, off + m0: off + m0 + mc]
                    for hh in range(NHALVES):
                        nsl = slice(hh * NH, (hh + 1) * NH)
                        nc.tensor.matmul(z[:, nsl], lhsT, rt[:, nsl],
                                         start=(ki == 0),
                                         stop=(ki == len(cur) - 1))
                if probe == "mmonly":
                    base = cur[0][0]
                    new.append((base[0:mc], mc))
                    continue
                # output tile: lo chunks land in a rotating [73, N] tile
                # whose row 72 is a persistent 1.0 (next layer's bias row)
                if mi == 1:
                    full = hlo_tiles[hlo_ctr[0] % NLO]
                    hlo_ctr[0] += 1
                    ht = full[0:72]
                    out_entry = (full, 73)
                else:
                    ht = hpool.tile([mc, QUANT], fp16, tag=f"h{mi}")
                    out_entry = (ht, 128)
                et = epool.tile([mc, QUANT], fp16, tag=f"e{mi}")
                if li == 0:
                    # r0 has no ones-row in its rhs: 3-op path with bias APs
                    nc.scalar.activation(et[:], z[:], Act.Exp,
                                         bias=bias_ap(lname, chunk, "e", mc))
                    ut = epool.tile([mc, QUANT], fp16, tag=f"u{mi}")
                    nc.vector.tensor_scalar(ut[:], et[:], 1.0, None, Alu.min)
                    nc.vector.scalar_tensor_tensor(
                        ht[:], z[:], bias_ap(lname, chunk, "c", mc), ut[:],
                        Alu.add, Alu.max)
                else:
                    # z already contains zb+1 (bias ones-row in the matmul):
                    #   e  = Exp(z - 1);  h' = (e min 1) max z
                    nc.scalar.activation(et[:], z[:], Act.Exp,
                                         bias=bias_ap("m1", "hi", "e", mc))
                    nc.vector.scalar_tensor_tensor(
                        ht[:], et[:], 1.0, z[:], Alu.min, Alu.max)
                new.append(out_entry)
            if lname == "r2" and probe != "nostores":
                # store r+1 (fp16); host subtracts the 1
                for mi, (ht, mc) in enumerate(new):
                    rows = 128 if mi == 0 else 72
                    nc.sync.dma_start(
                        r_d[mi * 128:mi * 128 + rows, col0:col0 + QUANT],
                        ht[0:rows, :])
            return new

        def emit_yf(s, cur):
            col0 = s * QUANT
            yo = opool.tile([1, QUANT], f32, tag="y")
            for hh in range(NHALVES):
                nsl = slice(hh * NH, (hh + 1) * NH)
                zy = pshi.tile([1, NH], f32, tag="z0")
                for ki, (rt, kc) in enumerate(cur):
                    off, k, m = wcols["o_hi" if ki == 0 else "o_lo"]
                    nc.tensor.matmul(zy[:], wt[0:k, off:off + 1], rt[:, nsl],
                                     start=(ki == 0), stop=(ki == len(cur) - 1))
                # b_o rides the o_lo ones-row weight: plain copy out
                nc.scalar.copy(yo[:, nsl], zy[:])
            if probe != "nostores":
                nc.sync.dma_start(y_d[:, col0:col0 + QUANT], yo[:])

        def body():
            for w0 in range(0, n_tiles, wave):
                ss = list(range(w0, min(w0 + wave, n_tiles)))
                curs = {s: load_x(s) for s in ss}
                for li in range(6):
                    for s in ss:
                        curs[s] = emit_layer(s, li, curs[s])
                for s in ss:
                    emit_yf(s, curs[s])

        if reps == 1:
            body()
        else:
            with tc.For_i(0, reps, 1):
                body()

    nc.compile()
    return nc


def _route(t):
    """Host-side MoE routing: stable sort by t, even split over cores,
    pad each core's treat/control block up to a QUANT multiple."""
    order = np.argsort(t, kind="stable")
    n_t = int((t == 0).sum())
    n_c = t.shape[0] - n_t

    def cap(n):
        if n == 0:
            return 0
        return int(math.ceil(n / (NCORES * QUANT))) * QUANT

    NT, NC = cap(n_t), cap(n_c)
    tchunks, cchunks = [], []
    for c in range(NCORES):
        lo = min(c * NT, n_t)
        hi = min((c + 1) * NT, n_t)
        tchunks.append(order[lo:hi])
        lo = min(c * NC, n_c)
        hi = min((c + 1) * NC, n_c)
        cchunks.append(order[n_t + lo:n_t + hi])
    return NT, NC, tchunks, cchunks


def kernel(**inputs):
    x = np.asarray(inputs["x"], np.float32)
    t = np.asarray(inputs["t"]).astype(np.int64)
    assert x.shape == (B, FEAT)

    wpack, wcols, bpack, bcol_idx, b_o_eff = _pack_host(inputs)
    NT, NC, tchunks, cchunks = _route(t)
    R = NT + NC

    x16 = x.astype(_F16)
    in_maps = []
    for c in range(NCORES):
        xc = np.zeros((FEAT, R), _F16)
        tc_, cc_ = tchunks[c], cchunks[c]
        if len(tc_):
            xc[:, :len(tc_)] = x16[tc_].T
        if len(cc_):
            xc[:, NT:NT + len(cc_)] = x16[cc_].T
        in_maps.append({"xT": np.ascontiguousarray(xc),
                        "wpack": wpack, "bpack": bpack})

    nc = _build_program(R, NT // QUANT, wcols, bcol_idx, b_o_eff,
                        wpack.shape[1], bpack.shape[1])

    from concourse import bass_utils
    res = bass_utils.run_bass_kernel_spmd(
        nc, in_maps, list(range(NCORES))).results

    r_full = np.empty((B, REP), np.float32)
    yf_full = np.empty((B, 1), np.float32)
    for c in range(NCORES):
        rT = res[c]["rT"]
        yf = res[c]["yf"]
        tc_, cc_ = tchunks[c], cchunks[c]
        if len(tc_):
            r_full[tc_] = rT[:, :len(tc_)].T.astype(np.float32) - 1.0
            yf_full[tc_, 0] = yf[0, :len(tc_)]
        if len(cc_):
            r_full[cc_] = rT[:, NT:NT + len(cc_)].T.astype(np.float32) - 1.0
            yf_full[cc_, 0] = yf[0, NT:NT + len(cc_)]
    return (r_full, yf_full)


if __name__ == "__main__":
    import reference

    inputs = {k: np.asarray(v) for k, v in reference.setup_inputs().items()}
    out = kernel(**inputs)
    print("r", out[0].shape, out[0].dtype, "yf", out[1].shape, out[1].dtype)


# revision 35
# speedup vs baseline: 17.9051x; 2.6697x over previous
"""CFR network (moe_routing) Trainium2 Bass kernel.

Strategy:
  - Pure data parallel over 8 NeuronCores; MoE routing (treat/control expert
    selection) is done host-side by stable-sorting rows on t so each core
    processes a contiguous treat block then a contiguous control block and
    only ever runs ONE expert MLP per row (halves expert compute vs the
    dense reference).
  - Feature-major activations on chip: weights are the stationary matmul
    operand, activations stream as rhs [features(K) x batch(N)].  Everything
    is fp16 (matmuls accumulate fp32 into PSUM).
  - Layers carry h' = elu(z)+1 (the -1 is folded into the next layer's bias,
    b_eff = b - W @ 1), computed exactly as
        e  = Exp(z + b)            ScalarE, PSUM read, fused bias
        u  = min(e, 1)             DVE 4x / GpSimd, SBUF fp16
        h' = (z + (b+1)) max u     DVE scalar_tensor_tensor, PSUM read
  - Emission is software-pipelined: layers of a wave of supertiles are
    interleaved so each in-order engine always has independent work.
  - The r output is stored as h' (= r+1, fp16); the host subtracts 1.
"""

import math
from contextlib import ExitStack

import numpy as np

B = 262144
FEAT = 128
REP = 200
HYP = 200
NCORES = 8
QUANT = 1024  # batch columns per supertile
NH = 512      # matmul free-dim per instruction (one PSUM bank of fp32)

_F16 = np.float16

_LAYERS = ["r0", "r1", "r2", "t0", "t1", "t2", "c0", "c1", "c2"]


def _pack_host(inputs):
    """Transpose/cast weights, fold the +1 carry into biases, pack into two
    flat arrays (one fp16 weight pack, one fp32 bias pack)."""
    ws = {}
    bs = {}
    for name in _LAYERS:
        w = np.asarray(inputs[f"w_{name[0]}{name[1]}"], np.float32)
        b = np.asarray(inputs[f"b_{name[0]}{name[1]}"], np.float32)
        if name != "r0":
            b = b - w.sum(axis=1)
        ws[name] = w.T.astype(_F16)  # [d_in, d_out] = lhsT
        bs[name] = b

    w_o = np.asarray(inputs["w_o"], np.float32)
    b_o = np.asarray(inputs["b_o"], np.float32)
    b_o_eff = float(b_o[0] - w_o.sum())
    wsT_o = w_o.T.astype(_F16)  # [200, 1]

    cols = []
    wcols = {}
    off = 0

    def add(name, arr):
        nonlocal off
        k, m = arr.shape
        pad = np.zeros((128, m), _F16)
        pad[:k] = arr
        cols.append(pad)
        wcols[name] = (off, k, m)
        off += m

    add("r0", ws["r0"])
    add("r0b", (bs["r0"] + 1.0)[None, :].astype(_F16))  # [1, 200] bias row
    for name in _LAYERS[1:]:
        add(name + "_hi", ws[name][:128])
        # lo K-chunk carries an extra ones-row weight = b_eff + 1, so the
        # matmul output lands in PSUM already biased (z = zb + 1)
        lo = np.concatenate(
            [ws[name][128:], (bs[name] + 1.0)[None, :].astype(_F16)], axis=0)
        add(name + "_lo", lo)
    add("o_hi", wsT_o[:128])
    o_lo = np.concatenate(
        [wsT_o[128:], np.array([[b_o_eff]], _F16)], axis=0)
    add("o_lo", o_lo)
    wpack = np.concatenate(cols, axis=1)

    bcols = []
    bcol_idx = {}
    for name in _LAYERS:
        b = bs[name]
        for chunk, sl in (("hi", slice(0, 128)), ("lo", slice(128, 200))):
            for var, delta in (("e", 0.0), ("c", 1.0)):
                v = np.zeros((128,), np.float32)
                seg = b[sl] + delta
                v[: len(seg)] = seg
                bcol_idx[(name, chunk, var)] = len(bcols)
                bcols.append(v)
    v = np.zeros((128,), np.float32)
    v[0] = b_o_eff
    bcol_idx[("o", "hi", "e")] = len(bcols)
    bcols.append(v)
    bcol_idx[("m1", "hi", "e")] = len(bcols)
    bcols.append(np.full((128,), -1.0, np.float32))
    bpack = np.stack(bcols, axis=1).astype(np.float32)

    return wpack, wcols, bpack, bcol_idx, b_o_eff


def _build_program(R, n_treat_tiles, wcols, bcol_idx, b_o_eff, wcols_total, nb,
                   reps=1, wave=3,
                   umin_pool=lambda li, mi: False,
                   path_c=lambda li, mi: mi == 1 and li in (1, 3, 5),
                   probe=None):
    """Single-core Bass/Tile program (shared SPMD across the 8 cores).

    reps>1 repeats the whole computation inside the NEFF (timing only)."""
    import concourse.bass as bass
    import concourse.tile as tile
    from concourse import bacc, mybir

    fp16 = mybir.dt.float16
    f32 = mybir.dt.float32
    Alu = mybir.AluOpType
    Act = mybir.ActivationFunctionType

    nc = bacc.Bacc("TRN2", target_bir_lowering=False, debug=False,
                   enable_asserts=False)

    x_d = nc.dram_tensor("xT", [FEAT, R], fp16, kind="ExternalInput")
    w_d = nc.dram_tensor("wpack", [128, wcols_total], fp16, kind="ExternalInput")
    b_d = nc.dram_tensor("bpack", [128, nb], f32, kind="ExternalInput")
    r_d = nc.dram_tensor("rT", [REP, R], fp16, kind="ExternalOutput")
    y_d = nc.dram_tensor("yf", [1, R], f32, kind="ExternalOutput")

    n_tiles = R // QUANT
    NHALVES = QUANT // NH

    with tile.TileContext(nc) as tc, ExitStack() as ctx:
        wpool = ctx.enter_context(tc.tile_pool(name="w", bufs=1))
        xpool = ctx.enter_context(tc.tile_pool(name="x", bufs=8))
        hpool = ctx.enter_context(tc.tile_pool(name="h", bufs=6))
        epool = ctx.enter_context(tc.tile_pool(name="e", bufs=6))
        opool = ctx.enter_context(tc.tile_pool(name="o", bufs=4))
        pshi = ctx.enter_context(
            tc.tile_pool(name="pshi", bufs=2, space=bass.MemorySpace.PSUM))
        pslo = ctx.enter_context(
            tc.tile_pool(name="pslo", bufs=2, space=bass.MemorySpace.PSUM))

        wt = wpool.tile([128, wcols_total], fp16, tag="wt")
        nc.sync.dma_start(wt[:], w_d[:])
        bt = wpool.tile([128, nb], f32, tag="bt")
        nc.sync.dma_start(bt[:], b_d[:])
        dummy = None
        if probe == "noact":
            dummy = []
            for mi, mc in ((0, 128), (1, 72)):
                dtile = wpool.tile([mc, QUANT], fp16, tag=f"dum{mi}")
                nc.vector.memset(dtile[:], 1.0)
                dummy.append(dtile)

        # manually rotated lo-activation tiles [73, QUANT]: row 72 is a
        # persistent 1.0 (feeds the bias ones-row of the next matmul)
        NLO = 9
        hlo_tiles = []
        for i in range(NLO):
            hlo = wpool.tile([73, QUANT], fp16, tag=f"hlo{i}")
            # engine APs need a 32-aligned partition base: set rows 64-72 to
            # 1.0; rows 64-71 are rewritten by every stt, row 72 persists
            nc.vector.memset(hlo[64:73, :], 1.0)
            hlo_tiles.append(hlo)
        hlo_ctr = [0]
        # ones row for r0's bias matmul (K=1 chunk)
        ones_t = wpool.tile([1, QUANT], fp16, tag="ones_t")
        nc.vector.memset(ones_t[:], 1.0)

        def bias_ap(name, chunk, var, parts):
            c = bcol_idx[(name, chunk, var)]
            return bt[0:parts, c:c + 1]

        def load_x(s):
            col0 = s * QUANT
            xt = xpool.tile([FEAT, QUANT], fp16, tag="x")
            nc.sync.dma_start(xt[:], x_d[:, col0:col0 + QUANT])
            return [(xt, FEAT), (ones_t, 1)]

        def emit_layer(s, li, cur):
            expert = "t" if s < n_treat_tiles else "c"
            lname = ["r0", "r1", "r2", expert + "0", expert + "1",
                     expert + "2"][li]
            col0 = s * QUANT
            wchunks = (["r0", "r0b"] if lname == "r0"
                       else [lname + "_hi", lname + "_lo"])
            new = []
            for mi, (mc, m0, chunk, pool) in enumerate(
                    ((128, 0, "hi", pshi), (72, 128, "lo", pslo))):
                z = pool.tile([mc, QUANT], f32, tag=f"z{mi}")
                # K-chunk outer so each stationary weight load serves all
                # N-halves (halves the LDWEIGHTS count)
                for ki, (rt, kc) in enumerate(cur):
                    off, k, m = wcols[wchunks[ki]]
                    lhsT = wt[0:k, off + m0: off + m0 + mc]
                    for hh in range(NHALVES):
                        nsl = slice(hh * NH, (hh + 1) * NH)
                        nc.tensor.matmul(z[:, nsl], lhsT, rt[:, nsl],
                                         start=(ki == 0),
                                         stop=(ki == len(cur) - 1))
                if probe == "mmonly":
                    base = cur[0][0]
                    rows = 73 if mi == 1 else 128
                    new.append((base[0:rows], rows))
                    continue
                # output tile: lo chunks land in a rotating [73, N] tile
                # whose row 72 is a persistent 1.0 (next layer's bias row)
                if mi == 1:
                    full = hlo_tiles[hlo_ctr[0] % NLO]
                    hlo_ctr[0] += 1
                    ht = full[0:72]
                    out_entry = (full, 73)
                else:
                    ht = hpool.tile([mc, QUANT], fp16, tag=f"h{mi}")
                    out_entry = (ht, 128)
                # z already contains zb+1 (bias rides the matmul):
                #   e  = Exp(z - 1);  h' = (e min 1) max z
                et = epool.tile([mc, QUANT], fp16, tag=f"e{mi}")
                nc.scalar.activation(et[:], z[:], Act.Exp,
                                     bias=bias_ap("m1", "hi", "e", mc))
                nc.vector.scalar_tensor_tensor(
                    ht[:], et[:], 1.0, z[:], Alu.min, Alu.max)
                new.append(out_entry)
            if lname == "r2" and probe != "nostores":
                # store r+1 (fp16); host subtracts the 1
                for mi, (ht, mc) in enumerate(new):
                    rows = 128 if mi == 0 else 72
                    nc.sync.dma_start(
                        r_d[mi * 128:mi * 128 + rows, col0:col0 + QUANT],
                        ht[0:rows, :])
            return new

        def emit_yf(s, cur):
            col0 = s * QUANT
            yo = opool.tile([1, QUANT], f32, tag="y")
            zy = pshi.tile([1, QUANT], f32, tag="z0")
            for ki, (rt, kc) in enumerate(cur):
                off, k, m = wcols["o_hi" if ki == 0 else "o_lo"]
                for hh in range(NHALVES):
                    nsl = slice(hh * NH, (hh + 1) * NH)
                    nc.tensor.matmul(zy[:, nsl], wt[0:k, off:off + 1],
                                     rt[:, nsl],
                                     start=(ki == 0), stop=(ki == len(cur) - 1))
            # b_o rides the o_lo ones-row weight: plain copy out
            nc.scalar.copy(yo[:], zy[:])
            if probe != "nostores":
                nc.sync.dma_start(y_d[:, col0:col0 + QUANT], yo[:])

        def body():
            for w0 in range(0, n_tiles, wave):
                ss = list(range(w0, min(w0 + wave, n_tiles)))
                curs = {s: load_x(s) for s in ss}
                for li in range(6):
                    for s in ss:
                        curs[s] = emit_layer(s, li, curs[s])
                for s in ss:
                    emit_yf(s, curs[s])

        if reps == 1:
            body()
        else:
            with tc.For_i(0, reps, 1):
                body()

    nc.compile()
    return nc


def _route(t):
    """Host-side MoE routing: stable sort by t, even split over cores,
    pad each core's treat/control block up to a QUANT multiple."""
    order = np.argsort(t, kind="stable")
    n_t = int((t == 0).sum())
    n_c = t.shape[0] - n_t

    def cap(n):
        if n == 0:
            return 0
        return int(math.ceil(n / (NCORES * QUANT))) * QUANT

    NT, NC = cap(n_t), cap(n_c)
    tchunks, cchunks = [], []
    for c in range(NCORES):
        lo = min(c * NT, n_t)
        hi = min((c + 1) * NT, n_t)
        tchunks.append(order[lo:hi])
        lo = min(c * NC, n_c)
        hi = min((c + 1) * NC, n_c)
        cchunks.append(order[n_t + lo:n_t + hi])
    return NT, NC, tchunks, cchunks


def kernel(**inputs):
    x = np.asarray(inputs["x"], np.float32)
    t = np.asarray(inputs["t"]).astype(np.int64)
    assert x.shape == (B, FEAT)

    wpack, wcols, bpack, bcol_idx, b_o_eff = _pack_host(inputs)
    NT, NC, tchunks, cchunks = _route(t)
    R = NT + NC

    x16 = x.astype(_F16)
    in_maps = []
    for c in range(NCORES):
        xc = np.zeros((FEAT, R), _F16)
        tc_, cc_ = tchunks[c], cchunks[c]
        if len(tc_):
            xc[:, :len(tc_)] = x16[tc_].T
        if len(cc_):
            xc[:, NT:NT + len(cc_)] = x16[cc_].T
        in_maps.append({"xT": np.ascontiguousarray(xc),
                        "wpack": wpack, "bpack": bpack})

    nc = _build_program(R, NT // QUANT, wcols, bcol_idx, b_o_eff,
                        wpack.shape[1], bpack.shape[1])

    from concourse import bass_utils
    res = bass_utils.run_bass_kernel_spmd(
        nc, in_maps, list(range(NCORES))).results

    r_full = np.empty((B, REP), np.float32)
    yf_full = np.empty((B, 1), np.float32)
    for c in range(NCORES):
        rT = res[c]["rT"]
        yf = res[c]["yf"]
        tc_, cc_ = tchunks[c], cchunks[c]
        if len(tc_):
            r_full[tc_] = rT[:, :len(tc_)].T.astype(np.float32) - 1.0
            yf_full[tc_, 0] = yf[0, :len(tc_)]
        if len(cc_):
            r_full[cc_] = rT[:, NT:NT + len(cc_)].T.astype(np.float32) - 1.0
            yf_full[cc_, 0] = yf[0, NT:NT + len(cc_)]
    return (r_full, yf_full)


if __name__ == "__main__":
    import reference

    inputs = {k: np.asarray(v) for k, v in reference.setup_inputs().items()}
    out = kernel(**inputs)
    print("r", out[0].shape, out[0].dtype, "yf", out[1].shape, out[1].dtype)
